# revision 1
# baseline (speedup 1.0000x reference)
"""DNC Trainium2 Bass kernel — pure data-parallel over 8 NeuronCores (4 batch each).

Strategy (per core, Bc=4 batch elements):
  - All weights + recurrent state SBUF-resident across the whole T=128 scan.
  - Controller matmuls: activations-stationary (lhsT [K,4]) streaming fp32 weights.
  - Usage/alloc/link state in "T layout" [128 n-part, (chunk, b) cols]; row-vector
    broadcasts built with PE rank-1 matmuls (ones x single-partition row).
  - Allocation (usage argsort) = pairwise compare (tie-broken by index via key
    perturbation) + log-domain product via PE matmul. Validated in mirror.py.
  - Link diagonal left unmasked; tracked separately (ddiag) and subtracted from
    the fw/bw reads (link diag never feeds anything else).
"""
import sys
from contextlib import ExitStack

import numpy as np

sys.path.insert(0, "/opt/trn_rl_repo")

import bass_rust  # noqa: E402
import concourse.bass as bass  # noqa: E402
import concourse.tile as tile  # noqa: E402
from concourse import mybir  # noqa: E402
from concourse.bass_utils import run_bass_kernel_spmd  # noqa: E402

N, WM, R, H, D = 256, 64, 4, 512, 512
EPS = 1e-6
IF = 471
T_FULL, B_FULL, NCORES = 128, 32, 8
LAST_EXEC_NS = None
FP = mybir.dt.float32
AL = mybir.AluOpType
AF = mybir.ActivationFunctionType
AX = mybir.AxisListType


def _perm_iface_cols():
    off = {}
    o = 0
    for name, sz in [("read_keys", R * WM), ("read_str", R), ("write_key", WM),
                     ("write_str", 1), ("erase", WM), ("write_vec", WM),
                     ("free", R), ("alloc_g", 1), ("write_g", 1), ("modes", 3 * R)]:
        off[name] = (o, o + sz); o += sz
    order = ["read_keys", "write_key", "write_vec", "erase", "free",
             "alloc_g", "write_g", "read_str", "write_str", "modes"]
    return np.concatenate([np.arange(*off[n]) for n in order])


def build(nc, T, Bc):
    NB = 8 * 2048 + 4 * 2048 + 4 * IF + 4 * 512 + 4 * 512 + 16 + 128 + 1024 + 4 * T * Bc
    BL = nc.dram_tensor("blob", [128, NB], FP, kind="ExternalInput").ap()
    outs = nc.dram_tensor("outs", [T, Bc, D], FP, kind="ExternalOutput").ap()

    with tile.TileContext(nc, trace_sim=False, linearize=True) as tc, ExitStack() as ctx:
        P = ctx.enter_context(tc.tile_pool(name="state", bufs=1))
        PP = ctx.enter_context(tc.tile_pool(name="ps", bufs=6, space="PSUM"))
        PB = ctx.enter_context(tc.tile_pool(name="psbig", bufs=1, space="PSUM"))
        PS = ctx.enter_context(tc.tile_pool(name="scr", bufs=1))

        f32 = FP
        blob = P.tile([128, NB], f32)
        nc.sync.dma_start(blob[:], BL[:])
        o = 0
        def blk(cols, rows=128):
            nonlocal o
            ap = blob[0:rows, o:o + cols]; o += cols; return ap
        Wg_s = blk(8 * 2048)
        Wgr_s = blk(4 * 2048, rows=64)
        Wif_s = blk(4 * IF)
        Wp_s = blk(4 * 512)
        Wr_s = blk(4 * 512, rows=64)
        CC = blk(16)
        IDT = blk(128)
        ES = blk(1024, rows=8)
        embT_s = blk(4 * T * Bc)

        _ztc = [0]
        def zt(shape, val=0.0):
            _ztc[0] += 1
            t = P.tile(shape, f32, name=f"st{_ztc[0]}")
            nc.vector.memset(t[:], val); return t

        hT = zt([128, 16])            # [h%128, (kc4, b)]
        c_s = zt([4, 512])
        L = [zt([128, 1024]), zt([128, 1024])]   # [i, (b, j256)] b-major
        G = [zt([128, 1024]), zt([128, 1024])]   # [q, (b, s256)] = L^T
        M = [zt([128, 256], EPS), zt([128, 256], EPS)]  # [n, (b, w64)]
        MT = zt([64, 1024], EPS)      # [w, (b, n256)]
        uT = zt([128, 8])             # [n%128, (c2, b4)] c-major
        wwT = zt([128, 8])
        precT = zt([128, 8])
        ddiag = zt([128, 8])
        rwt = [zt([128, 16]), zt([128, 16])]  # [n%128, (b, r)] b-major, per n-chunk
        rvT = zt([64, 16])            # [w, (b, r)] b-major
        rinvMpre = zt([128, 8], 1.0 / (EPS * np.sqrt(WM)))  # rinvM of pre-write M (carried)

        ONES = CC[:, 5:6]

        def rank1(ps_ap, src_ap, r, acc=False, stop=True):
            # broadcast row r of src_ap [nrows, C] to all 128 partitions of ps_ap
            nr = src_ap.shape[0]
            nc.tensor.matmul(ps_ap, ES[0:nr, r * 128:r * 128 + 128], src_ap,
                             start=not acc, stop=stop)

        def s0(ap2, reps, inner):  # [P, k] -> [P, k*? ] free step-0 broadcast
            return ap2.unsqueeze(2).broadcast_to(list(ap2.shape) + [inner]) if reps is None else None

        for t in range(T):
            # ===== controller gates [4b, 2048] in two 1024-col halves
            sig = PS.tile([4, 1536], f32, tag="sig")
            tg = PS.tile([4, 512], f32, tag="tg")
            for hf in range(2):
                gps = PB.tile([4, 1024], f32, tag="big", name="gates")
                first = True
                for kc in range(4):
                    lhs = embT_s[:, (kc * T + t) * Bc:(kc * T + t) * Bc + Bc]
                    for nk in range(2):
                        nko = hf * 2 + nk
                        nc.tensor.matmul(gps[:, nk * 512:(nk + 1) * 512], lhs,
                                         Wg_s[:, kc * 2048 + nko * 512:kc * 2048 + (nko + 1) * 512],
                                         start=first, stop=False)
                    first = False
                for r in range(4):
                    lhs = rvT[:].rearrange("p (b r) -> p r b", r=4)[:, r, :]  # [64, 4b]
                    for nk in range(2):
                        nko = hf * 2 + nk
                        nc.tensor.matmul(gps[:, nk * 512:(nk + 1) * 512], lhs,
                                         Wgr_s[:, r * 2048 + nko * 512:r * 2048 + (nko + 1) * 512],
                                         start=False, stop=False)
                for kc in range(4):
                    lhs = hT[:, kc * 4:kc * 4 + 4]
                    for nk in range(2):
                        nko = hf * 2 + nk
                        nc.tensor.matmul(gps[:, nk * 512:(nk + 1) * 512], lhs,
                                         Wg_s[:, (4 + kc) * 2048 + nko * 512:(4 + kc) * 2048 + (nko + 1) * 512],
                                         start=False, stop=(kc == 3))
                if hf == 0:
                    nc.scalar.activation(sig[:, 0:1024], gps[:], AF.Sigmoid)
                else:
                    nc.scalar.activation(sig[:, 1024:1536], gps[:, 0:512], AF.Sigmoid)
                    nc.scalar.activation(tg[:], gps[:, 512:1024], AF.Tanh)
            tA = PS.tile([4, 512], f32, tag="lstA")
            nc.vector.tensor_mul(tA[:], sig[:, 0:512], tg[:])
            tB = PS.tile([4, 512], f32, tag="lstB")
            nc.vector.tensor_mul(tB[:], sig[:, 512:1024], c_s[:])
            nc.vector.tensor_add(c_s[:], tA[:], tB[:])
            tcs = PS.tile([4, 512], f32, tag="lstC")
            nc.scalar.activation(tcs[:], c_s[:], AF.Tanh)
            h_s = PS.tile([4, 512], f32, tag="h")
            nc.vector.tensor_mul(h_s[:], sig[:, 1024:1536], tcs[:])
            hTp = PP.tile([128, 16], f32, tag="pp", name="hT")
            for kc in range(4):
                nc.tensor.transpose(hTp[:, kc * 4:kc * 4 + 4], h_s[:, kc * 128:(kc + 1) * 128], IDT[:4, :4])
            nc.vector.tensor_copy(hT[:], hTp[:])

            # ===== interface z^T (cols (mc, b)); chunk3 rows: erase64|free4|ag|wg|rs4|ws|modes12
            zTp = PP.tile([128, 16], f32, tag="pp", name="zT")
            for mc in range(4):
                mw = 128 if mc < 3 else 87
                for kc in range(4):
                    nc.tensor.matmul(zTp[:mw, mc * 4:mc * 4 + 4],
                                     Wif_s[:, kc * IF + mc * 128:kc * IF + mc * 128 + mw],
                                     hT[:, kc * 4:kc * 4 + 4], start=(kc == 0), stop=(kc == 3))
            zTs = PS.tile([128, 16], f32, tag="zTs")
            nc.vector.tensor_copy(zTs[:, 0:12], zTp[:, 0:12])
            nc.vector.tensor_copy(zTs[0:87, 12:16], zTp[0:87, 12:16])
            # row-layout chunk3 [4b, 87]
            z3p = PP.tile([4, 128], f32, tag="pp", name="z3p")
            nc.tensor.transpose(z3p[:, 0:87], zTs[0:87, 12:16], IDT[:87, :87])
            z3r = PS.tile([4, 87], f32, tag="z3r")
            nc.vector.tensor_copy(z3r[:], z3p[:, 0:87])
            # row-layout write_vec (+wk unused) [4b, 128]
            z2p = PP.tile([4, 128], f32, tag="pp", name="z2p")
            nc.tensor.transpose(z2p[:], zTs[:, 8:12], IDT[:])
            wv_r = PS.tile([4, 128], f32, tag="wvr")
            nc.vector.tensor_copy(wv_r[:], z2p[:])   # [:, 64:128] = write_vec row
            # gather all keys to base partitions 0-63: keyg [64, (head5, b)]
            kgp = PP.tile([64, 20], f32, tag="pp", name="kgp")
            for c in range(2):   # read-key chunks
                zcp = PP.tile([4, 128], f32, tag="pp", name="zcp")
                nc.tensor.transpose(zcp[:], zTs[:, c * 4:(c + 1) * 4], IDT[:])
                zcs = PS.tile([4, 128], f32, tag="zcs")
                nc.vector.tensor_copy(zcs[:], zcp[:])
                for hh in range(2):
                    nc.tensor.transpose(kgp[:, (c * 2 + hh) * 4:(c * 2 + hh) * 4 + 4],
                                        zcs[:, hh * 64:(hh + 1) * 64], IDT[:4, :4])
            nc.tensor.transpose(kgp[:, 16:20], wv_r[:, 0:64], IDT[:4, :4])  # write key
            keyg = PS.tile([64, 20], f32, tag="keyg")
            nc.vector.tensor_copy(keyg[:], kgp[:])
            sg_r = PS.tile([4, 70], f32, tag="sgr")  # sigma(erase|free|ag|wg) rows
            nc.scalar.activation(sg_r[:], z3r[:, 0:70], AF.Sigmoid)
            spr = PS.tile([4, 5], f32, tag="spr")    # beta = 1+ln(1+exp(.)) [rs4, ws]
            nc.scalar.activation(spr[:], z3r[:, 70:75], AF.Exp)
            nc.vector.tensor_scalar(spr[:], spr[:], 1.0, None, AL.add)
            nc.scalar.activation(spr[:], spr[:], AF.Ln)
            nc.vector.tensor_scalar(spr[:], spr[:], 1.0, None, AL.add)
            em = PS.tile([4, 12], f32, tag="em")     # modes softmax [(r,m)]
            nc.scalar.activation(em[:], z3r[:, 75:87], AF.Exp)
            ems = PS.tile([4, 4], f32, tag="ems")
            nc.vector.tensor_reduce(ems[:], em[:].rearrange("p (r m) -> p r m", m=3), AX.X, AL.add)
            nc.vector.reciprocal(ems[:], ems[:])
            nc.vector.tensor_mul(em[:].rearrange("p (r m) -> p r m", m=3),
                                 em[:].rearrange("p (r m) -> p r m", m=3),
                                 ems[:].unsqueeze(2).broadcast_to([4, 4, 3]))

            # ===== retention & usage (T layout)
            fB = PP.tile([128, 16], f32, tag="pp", name="fB")   # [n%128?, (b, r)] value = sigma(free)[b, r]
            for b in range(4):
                rank1(fB[:, b * 4:b * 4 + 4], sg_r[:, 64:68], b)
            ret = PS.tile([128, 8], f32, tag="ret")
            for ic in range(2):
                q = PS.tile([128, 16], f32, tag="retq")
                nc.vector.tensor_mul(q[:], rwt[ic][:], fB[:])
                nc.vector.tensor_scalar(q[:], q[:], -1.0, 1.0, AL.mult, AL.add)
                q4 = q[:].rearrange("p (b r) -> p b r", r=4)
                q2 = PS.tile([128, 8], f32, tag="retq2")
                nc.vector.tensor_mul(q2[:].rearrange("p (b r) -> p b r", r=2),
                                     q4[:, :, 0:2], q4[:, :, 2:4])
                q22 = q2[:].rearrange("p (b r) -> p b r", r=2)
                nc.vector.tensor_mul(ret[:, ic * 4:ic * 4 + 4].unsqueeze(2),
                                     q22[:, :, 0:1], q22[:, :, 1:2])
            tm8 = PS.tile([128, 8], f32, tag="tm8")
            nc.vector.tensor_scalar(tm8[:], uT[:], -1.0, 1.0, AL.mult, AL.add)  # 1-u
            nc.vector.tensor_mul(tm8[:], wwT[:], tm8[:])
            nc.vector.tensor_add(tm8[:], tm8[:], uT[:])
            nc.vector.tensor_mul(uT[:], tm8[:], ret[:])

            # ===== allocation
            keysT = PS.tile([128, 8], f32, tag="keysT")
            paT = CC[:, 0:2].unsqueeze(2).broadcast_to([128, 2, 4])
            pbT = CC[:, 2:4].unsqueeze(2).broadcast_to([128, 2, 4])
            k3 = keysT[:].rearrange("p (c b) -> p c b", b=4)
            nc.vector.tensor_mul(k3, uT[:].rearrange("p (c b) -> p c b", b=4), paT)
            nc.vector.tensor_add(k3, k3, pbT)
            luT = PS.tile([128, 8], f32, tag="luT")
            nc.vector.tensor_scalar(luT[:], uT[:], 1e-38, None, AL.max)
            nc.scalar.activation(luT[:], luT[:], AF.Ln)
            kfp = PP.tile([8, 128], f32, tag="pp", name="kfp")
            nc.tensor.transpose(kfp[:], keysT[:], IDT[:])
            kfs = PS.tile([8, 128], f32, tag="kfs")
            nc.vector.tensor_copy(kfs[:], kfp[:])
            kb = PB.tile([128, 1024], f32, tag="big", name="kb")   # keys bcast [128, (b, n)]
            for b in range(4):
                for c in range(2):
                    rank1(kb[:, b * 256 + c * 128:b * 256 + (c + 1) * 128], kfs[:], c * 4 + b)
            exTs = []
            for mc in range(2):
                CT = PS.tile([128, 1024], f32, tag="CT")
                nc.vector.tensor_tensor(
                    CT[:].rearrange("p (b n) -> p b n", n=256), kb[:].rearrange("p (b n) -> p b n", n=256),
                    keysT[:, mc * 4:mc * 4 + 4].unsqueeze(2).broadcast_to([128, 4, 256]),
                    AL.is_gt)
                exT = PP.tile([128, 8], f32, tag="pp", name=f"exT{mc}")
                for b in range(4):
                    for nck in range(2):
                        nc.tensor.matmul(exT[:, nck * 4 + b:nck * 4 + b + 1],
                                         CT[:, b * 256 + nck * 128:b * 256 + (nck + 1) * 128],
                                         luT[:, mc * 4 + b:mc * 4 + b + 1],
                                         start=True, stop=True)
                exTs.append(exT)
            alT = PS.tile([128, 8], f32, tag="alT")
            nc.vector.tensor_copy(alT[:], exTs[0][:])
            nc.vector.tensor_add(alT[:], alT[:], exTs[1][:])
            nc.scalar.activation(alT[:], alT[:], AF.Exp)
            omu = PS.tile([128, 8], f32, tag="omu")
            nc.vector.tensor_scalar(omu[:], uT[:], -1.0, 1.0, AL.mult, AL.add)
            nc.vector.tensor_mul(alT[:], alT[:], omu[:])

            # ===== write content weight cw (pre-write M via MT/rinvMpre)
            cwp = PP.tile([128, 8], f32, tag="pp", name="cwp")   # simT [n%128, (c? no: (nc, b))] cols nc*4+b
            for b in range(4):
                lhs = keyg[:, 16 + b:17 + b]  # write key [64,1]
                for ncc in range(2):
                    nc.tensor.matmul(cwp[:, ncc * 4 + b:ncc * 4 + b + 1],
                                     MT[:, b * 256 + ncc * 128:b * 256 + (ncc + 1) * 128],
                                     lhs, start=True, stop=True)
            # scale by rinvMpre * (rinvK_wk * ws)
            sq = PS.tile([128, 12], f32, tag="sqk")
            nc.scalar.activation(sq[:], zTs[:, 0:12], AF.Square)
            kkp = PP.tile([4, 6], f32, tag="pp", name="kkp")
            for ch in range(3):
                nc.tensor.matmul(kkp[:, ch * 2:ch * 2 + 2], sq[:, ch * 4:ch * 4 + 4],
                                 CC[:, 6:8], start=True, stop=True)
            kk = PS.tile([4, 6], f32, tag="kk")   # cols [r0 r1 r2 r3 wk junk]
            nc.vector.tensor_scalar(kk[:], kkp[:], float(EPS * EPS), None, AL.max)
            nc.scalar.activation(kk[:], kk[:], AF.Ln)
            nc.scalar.activation(kk[:], kk[:], AF.Exp, scale=-0.5)  # 1/sqrt
            rb = PS.tile([4, 5], f32, tag="rbeta")  # [rs*rinvK r0..3, ws*rinvK wk]
            nc.vector.tensor_mul(rb[:], kk[:, 0:5], spr[:])
            # rinvM (pre) bcast and scales, softmax over n via ones-matmul
            sc1 = PP.tile([128, 8], f32, tag="pp", name="sc1")
            for b in range(4):
                for ncc in range(2):
                    rank1(sc1[:, ncc * 4 + b:ncc * 4 + b + 1], rb[:, 4:5], b)
            cws = PS.tile([128, 8], f32, tag="cws")
            nc.vector.tensor_mul(cws[:], cwp[:], rinvMpre[:])
            s1s = PS.tile([128, 8], f32, tag="s1s")
            nc.vector.tensor_copy(s1s[:], sc1[:])
            nc.vector.tensor_mul(cws[:], cws[:], s1s[:])
            nc.scalar.activation(cws[:], cws[:], AF.Exp)
            smp = PP.tile([1, 8], f32, tag="pp", name="smp")
            nc.tensor.matmul(smp[:, 0:4], ONES, cws[:, 0:4], start=True, stop=False)
            nc.tensor.matmul(smp[:, 4:8], ONES, cws[:, 4:8], start=False, stop=True)
            sms = PS.tile([1, 8], f32, tag="sms")
            nc.vector.tensor_copy(sms[:], smp[:])
            nc.vector.tensor_add(sms[:, 0:4], sms[:, 0:4], smp[:, 4:8])
            nc.vector.reciprocal(sms[:, 0:4], sms[:, 0:4])
            smb = PP.tile([128, 4], f32, tag="pp", name="smb")
            rank1(smb[:], sms[:, 0:4], 0)
            cwT = PS.tile([128, 8], f32, tag="cwT")   # cols (c, b)
            for c in range(2):
                nc.vector.tensor_mul(cwT[:, c * 4:c * 4 + 4], cws[:, c * 4:c * 4 + 4], smb[:])

            # ===== ww
            agB = PP.tile([128, 4], f32, tag="pp", name="agB")
            agwgp = PP.tile([2, 4], f32, tag="pp", name="agwg")
            nc.tensor.transpose(agwgp[:], sg_r[:, 68:70], IDT[:4, :4])
            agwg = PS.tile([2, 4], f32, tag="agwgs")
            nc.vector.tensor_copy(agwg[:], agwgp[:])
            rank1(agB[:], agwg[:], 0)
            wgB = PP.tile([128, 4], f32, tag="pp", name="wgB")
            rank1(wgB[:], agwg[:], 1)
            dT = PS.tile([128, 8], f32, tag="dT")
            nc.vector.tensor_sub(dT[:], alT[:], cwT[:])
            agBs = PS.tile([128, 4], f32, tag="agBs"); nc.vector.tensor_copy(agBs[:], agB[:])
            wgBs = PS.tile([128, 4], f32, tag="wgBs"); nc.vector.tensor_copy(wgBs[:], wgB[:])
            for c in range(2):
                nc.vector.tensor_mul(dT[:, c * 4:c * 4 + 4], dT[:, c * 4:c * 4 + 4], agBs[:])
            nc.vector.tensor_add(wwT[:], dT[:], cwT[:])
            for c in range(2):
                nc.vector.tensor_mul(wwT[:, c * 4:c * 4 + 4], wwT[:, c * 4:c * 4 + 4], wgBs[:])

            # ===== memory write M' = M(1-ww*e) + ww*v ; e/v bcasts [128, (b, w)]
            eBp = PB.tile([128, 256], f32, tag="big", name="eBp")
            for b in range(4):
                rank1(eBp[:, b * 64:(b + 1) * 64], sg_r[:, 0:64], b)
            eB = PS.tile([128, 256], f32, tag="eBs")
            nc.vector.tensor_copy(eB[:], eBp[:])
            vBp = PB.tile([128, 256], f32, tag="big", name="vBp")
            for b in range(4):
                rank1(vBp[:, b * 64:(b + 1) * 64], wv_r[:, 64:128], b)
            vB = PS.tile([128, 256], f32, tag="vBs")
            nc.vector.tensor_copy(vB[:], vBp[:])
            for ic in range(2):
                wws = wwT[:, ic * 4:ic * 4 + 4].unsqueeze(2).broadcast_to([128, 4, 64])
                q = PS.tile([128, 256], f32, tag="mq")
                nc.vector.tensor_mul(q[:].rearrange("p (b w) -> p b w", w=64),
                                     eB[:].rearrange("p (b w) -> p b w", w=64), wws)
                nc.vector.tensor_scalar(q[:], q[:], -1.0, 1.0, AL.mult, AL.add)
                nc.vector.tensor_mul(M[ic][:], M[ic][:], q[:])
                nc.vector.tensor_mul(q[:].rearrange("p (b w) -> p b w", w=64),
                                     vB[:].rearrange("p (b w) -> p b w", w=64), wws)
                nc.vector.tensor_add(M[ic][:], M[ic][:], q[:])
            # MT' via PE transposes
            for ic in range(2):
                for b in range(4):
                    mtp = PP.tile([64, 128], f32, tag="pp", name="mtp")
                    nc.tensor.transpose(mtp[:], M[ic][:, b * 64:(b + 1) * 64], IDT[:])
                    nc.vector.tensor_copy(MT[:, b * 256 + ic * 128:b * 256 + (ic + 1) * 128], mtp[:])
            # rinvM post-write (carried as rinvMpre for next step + used for cr now)
            msq = PS.tile([128, 256], f32, tag="msq")
            for ic in range(2):
                nc.scalar.activation(msq[:], M[ic][:], AF.Square)
                nc.vector.tensor_reduce(rinvMpre[:, ic * 4:ic * 4 + 4].unsqueeze(2),
                                        msq[:].rearrange("p (b w) -> p b w", w=64), AX.X, AL.add)
            nc.vector.tensor_scalar(rinvMpre[:], rinvMpre[:], float(EPS * EPS), None, AL.max)
            nc.scalar.activation(rinvMpre[:], rinvMpre[:], AF.Ln)
            nc.scalar.activation(rinvMpre[:], rinvMpre[:], AF.Exp, scale=-0.5)

            # ===== link update (L and G mirrored), flats of ww/prec
            wpf = PP.tile([8, 128], f32, tag="pp", name="wpf")
            nc.tensor.transpose(wpf[:], wwT[:], IDT[:])
            wpfs = PS.tile([8, 128], f32, tag="wpfs")
            nc.vector.tensor_copy(wpfs[:], wpf[:])
            ppf = PP.tile([8, 128], f32, tag="pp", name="ppf")
            nc.tensor.transpose(ppf[:], precT[:], IDT[:])
            ppfs = PS.tile([8, 128], f32, tag="ppfs")
            nc.vector.tensor_copy(ppfs[:], ppf[:])
            wBs = P.tile([128, 1024], f32, name="wBs")
            pBs = P.tile([128, 1024], f32, name="pBs")
            for half in range(2):
                wBp = PP.tile([128, 512], f32, tag="pp", name="wBp")
                pBp = PP.tile([128, 512], f32, tag="pp", name="pBp")
                for bb in range(2):
                    b = half * 2 + bb
                    for c in range(2):
                        rank1(wBp[:, bb * 256 + c * 128:bb * 256 + (c + 1) * 128], wpfs[:], c * 4 + b)
                        rank1(pBp[:, bb * 256 + c * 128:bb * 256 + (c + 1) * 128], ppfs[:], c * 4 + b)
                nc.vector.tensor_copy(wBs[:, half * 512:(half + 1) * 512], wBp[:])
                nc.vector.tensor_copy(pBs[:, half * 512:(half + 1) * 512], pBp[:])
            wB, pB = wBs[:], pBs[:]
            wwTm1 = PS.tile([128, 8], f32, tag="wwTm1")
            nc.vector.tensor_scalar(wwTm1[:], wwT[:], -1.0, None, AL.add)  # ww - 1? NO: ww + (-1)
            for ic in range(2):
                def b3(ap):
                    return ap.rearrange("p (b n) -> p b n", n=256)
                wi = wwTm1[:, ic * 4:ic * 4 + 4].unsqueeze(2).broadcast_to([128, 4, 256])
                wiP = wwT[:, ic * 4:ic * 4 + 4].unsqueeze(2).broadcast_to([128, 4, 256])
                piP = precT[:, ic * 4:ic * 4 + 4].unsqueeze(2).broadcast_to([128, 4, 256])
                Ft = PS.tile([128, 1024], f32, tag="Ft")
                nc.vector.tensor_add(b3(Ft[:]), b3(wB), wi)   # w_j + w_i - 1
                Tl = PS.tile([128, 1024], f32, tag="Tl")
                nc.gpsimd.tensor_mul(Tl[:], L[ic][:], Ft[:])      # L*(wi+wj-1)
                Ol = PS.tile([128, 1024], f32, tag="Ol")
                nc.vector.tensor_mul(b3(Ol[:]), b3(pB), wiP)   # w_i * p_j
                nc.vector.tensor_sub(L[ic][:], Ol[:], Tl[:])      # L' = O - T
                Tg = PS.tile([128, 1024], f32, tag="Tl", name="Tg")
                nc.gpsimd.tensor_mul(Tg[:], G[ic][:], Ft[:])
                Og = PS.tile([128, 1024], f32, tag="Ol", name="Og")
                nc.vector.tensor_mul(b3(Og[:]), b3(wB), piP)   # p_q * w_s
                nc.vector.tensor_sub(G[ic][:], Og[:], Tg[:])
            # ddiag' = ddiag*(1-2ww) + ww*prec
            d1 = PS.tile([128, 8], f32, tag="d1")
            nc.vector.tensor_scalar(d1[:], wwT[:], -2.0, 1.0, AL.mult, AL.add)
            nc.vector.tensor_mul(ddiag[:], ddiag[:], d1[:])
            nc.vector.tensor_mul(d1[:], wwT[:], precT[:])
            nc.vector.tensor_add(ddiag[:], ddiag[:], d1[:])
            # prec' = (1 - sum ww) * prec + ww
            swp = PP.tile([1, 8], f32, tag="pp", name="swp")
            nc.tensor.matmul(swp[:, 0:4], ONES, wwT[:, 0:4], start=True, stop=False)
            nc.tensor.matmul(swp[:, 4:8], ONES, wwT[:, 4:8], start=False, stop=True)
            sws = PS.tile([1, 8], f32, tag="sws")
            nc.vector.tensor_copy(sws[:], swp[:])
            nc.vector.tensor_add(sws[:, 0:4], sws[:, 0:4], sws[:, 4:8])
            sws = sws[0:1, 0:4] if False else sws
            nc.vector.tensor_scalar(sws[:, 0:4], sws[:, 0:4], -1.0, 1.0, AL.mult, AL.add)
            omsB = PP.tile([128, 4], f32, tag="pp", name="omsB")
            rank1(omsB[:], sws[:, 0:4], 0)
            omsBs = PS.tile([128, 4], f32, tag="omsBs")
            nc.vector.tensor_copy(omsBs[:], omsB[:])
            for c in range(2):
                nc.vector.tensor_mul(precT[:, c * 4:c * 4 + 4], precT[:, c * 4:c * 4 + 4], omsBs[:])
            nc.vector.tensor_add(precT[:], precT[:], wwT[:])

            # ===== read content cr (post-write M)
            crp = PP.tile([128, 16], f32, tag="pp", name="crp")  # [n%128? per nc-chunk separately]
            crp1 = PP.tile([128, 16], f32, tag="pp", name="crp1")
            for b in range(4):
                for r in range(4):
                    for ncc, dst in ((0, crp), (1, crp1)):
                        nc.tensor.matmul(dst[:, b * 4 + r:b * 4 + r + 1],
                                         MT[:, b * 256 + ncc * 128:b * 256 + (ncc + 1) * 128],
                                         keyg[:, r * 4 + b:r * 4 + b + 1], start=True, stop=True)
            rbb = PP.tile([128, 16], f32, tag="pp", name="rbb")
            for b in range(4):
                rank1(rbb[:, b * 4:b * 4 + 4], rb[:, 0:4], b)
            rbbs = PS.tile([128, 16], f32, tag="rbbs")
            nc.vector.tensor_copy(rbbs[:], rbb[:])
            crs = [PS.tile([128, 16], f32, tag="crs0", name="crs0"), PS.tile([128, 16], f32, tag="crs1", name="crs1")]
            for ncc, src in ((0, crp), (1, crp1)):
                nc.vector.tensor_mul(crs[ncc][:], src[:], rbbs[:])
                rm = rinvMpre[:, ncc * 4:ncc * 4 + 4].unsqueeze(2).broadcast_to([128, 4, 4])
                c3 = crs[ncc][:].rearrange("p (b r) -> p b r", r=4)
                nc.vector.tensor_mul(c3, c3, rm)
                nc.scalar.activation(crs[ncc][:], crs[ncc][:], AF.Exp)
            smr = PP.tile([1, 16], f32, tag="pp", name="smr")
            nc.tensor.matmul(smr[:], ONES, crs[0][:], start=True, stop=False)
            nc.tensor.matmul(smr[:], ONES, crs[1][:], start=False, stop=True)
            smrs = PS.tile([1, 16], f32, tag="smrs")
            nc.vector.reciprocal(smrs[:], smr[:])
            smrB = PP.tile([128, 16], f32, tag="pp", name="smrB")
            rank1(smrB[:], smrs[:], 0)
            smrBs = PS.tile([128, 16], f32, tag="smrBs")
            nc.vector.tensor_copy(smrBs[:], smrB[:])
            for ncc in range(2):
                nc.vector.tensor_mul(crs[ncc][:], crs[ncc][:], smrBs[:])

            # ===== fw/bw reads + diag correction + mode blend
            fwp, bwp = [], []
            for oc in range(2):
                fjc = [PP.tile([128, 16], f32, tag="pp", name=f"fw{oc}{j}") for j in range(2)]
                bjc = [PP.tile([128, 16], f32, tag="pp", name=f"bw{oc}{j}") for j in range(2)]
                for b in range(4):
                    for jc in range(2):
                        nc.tensor.matmul(fjc[jc][:, b * 4:b * 4 + 4],
                                         G[jc][:, b * 256 + oc * 128:b * 256 + (oc + 1) * 128],
                                         rwt[jc][:, b * 4:b * 4 + 4], start=True, stop=True)
                        nc.tensor.matmul(bjc[jc][:, b * 4:b * 4 + 4],
                                         L[jc][:, b * 256 + oc * 128:b * 256 + (oc + 1) * 128],
                                         rwt[jc][:, b * 4:b * 4 + 4], start=True, stop=True)
                fs = PS.tile([128, 16], f32, tag=f"fws{oc}", name=f"fws{oc}")
                nc.vector.tensor_copy(fs[:], fjc[0][:])
                nc.vector.tensor_add(fs[:], fs[:], fjc[1][:])
                bs = PS.tile([128, 16], f32, tag=f"bws{oc}", name=f"bws{oc}")
                nc.vector.tensor_copy(bs[:], bjc[0][:])
                nc.vector.tensor_add(bs[:], bs[:], bjc[1][:])
                fwp.append(fs); bwp.append(bs)
            emB = [PP.tile([128, 16], f32, tag="pp", name=f"emB{m}") for m in range(3)]
            for m in range(3):
                for b in range(4):
                    rank1(emB[m][:, b * 4:b * 4 + 4], em[:].rearrange("p (r m) -> p m r", m=3)[:, m, :], b)
            emBs = [PS.tile([128, 16], f32, tag=f"emBs{m}", name=f"emBs{m}") for m in range(3)]
            for m in range(3):
                nc.vector.tensor_copy(emBs[m][:], emB[m][:])
            for oc in range(2):
                dd = ddiag[:, oc * 4:oc * 4 + 4].unsqueeze(2).broadcast_to([128, 4, 4])
                corr = PS.tile([128, 16], f32, tag="corr")
                nc.vector.tensor_mul(corr[:].rearrange("p (b r) -> p b r", r=4),
                                     rwt[oc][:].rearrange("p (b r) -> p b r", r=4), dd)
                q1 = PS.tile([128, 16], f32, tag="bl1")
                nc.vector.tensor_sub(q1[:], fwp[oc][:], corr[:])
                nc.vector.tensor_mul(q1[:], q1[:], emBs[2][:])   # em2 * fw
                q2 = PS.tile([128, 16], f32, tag="bl2")
                nc.vector.tensor_sub(q2[:], bwp[oc][:], corr[:])
                nc.vector.tensor_mul(q2[:], q2[:], emBs[0][:])   # em0 * bw
                nc.vector.tensor_add(q1[:], q1[:], q2[:])
                nc.vector.tensor_mul(q2[:], crs[oc][:], emBs[1][:])
                nc.vector.tensor_add(rwt[oc][:], q1[:], q2[:])

            # ===== rvec^T [64, (b, r)] and outputs
            rvps = [PP.tile([64, 16], f32, tag="pp", name=f"rvp{j}") for j in range(2)]
            for b in range(4):
                for jc in range(2):
                    nc.tensor.matmul(rvps[jc][:, b * 4:b * 4 + 4],
                                     M[jc][:, b * 64:(b + 1) * 64],
                                     rwt[jc][:, b * 4:b * 4 + 4], start=True, stop=True)
            nc.vector.tensor_copy(rvT[:], rvps[0][:])
            nc.vector.tensor_add(rvT[:], rvT[:], rvps[1][:])
            ops = PB.tile([4, 512], f32, tag="big", name="outp")
            for kc in range(4):
                nc.tensor.matmul(ops[:], hT[:, kc * 4:kc * 4 + 4],
                                 Wp_s[:, kc * 512:(kc + 1) * 512], start=(kc == 0), stop=False)
            for r in range(4):
                nc.tensor.matmul(ops[:], rvT[:].rearrange("p (b r) -> p r b", r=4)[:, r, :],
                                 Wr_s[:, r * 512:(r + 1) * 512],
                                 start=False, stop=(r == 3))
            osb = PS.tile([4, 512], f32, tag="osb")
            nc.vector.tensor_copy(osb[:], ops[:])
            nc.sync.dma_start(outs[t], osb[:])

    _split_matmul_waits(nc)
    return outs


def _split_matmul_waits(nc):
    """Walrus's Matmult+LDW codegen supports a single sync-wait. Under
    linearize=True every instruction already waits on its total-order
    predecessor, which transitively covers all earlier data producers, so any
    additional data waits on a Matmult are redundant — keep only the wait that
    matches the immediate predecessor's update semaphore (the chain link)."""
    for fn in nc.m.functions:
        for blk in fn.blocks:
            insns = list(blk.instructions)
            for idx, ins in enumerate(insns):
                si = ins.sync_info
                if si is None or len(si.on_wait) <= 1:
                    continue
                waits = list(si.on_wait)
                keep = None
                if idx > 0:
                    prev = insns[idx - 1]
                    psi = prev.sync_info
                    upd = {u.ant_name for u in psi.on_update} if psi else set()
                    for w in waits:
                        if w.ant_name in upd:
                            keep = w
                            break
                if keep is None:
                    keep = waits[-1]
                import bass_rust as _br
                ins.sync_info = _br.SyncInfo(on_wait=[keep], on_update=list(si.on_update))


def _prep_core_inputs(emb_b, Wx, Wh, W_pre, W_iface, W_rout, T, Bc):
    """Host-side layout packing for one core. emb_b: (T, Bc, D)."""
    perm = _perm_iface_cols()
    Wif_p = W_iface[:, perm].astype(np.float32)
    # gate columns reordered [i | f | o | g] (sigmoid block contiguous)
    gperm = np.concatenate([np.arange(0, 512), np.arange(512, 1024),
                            np.arange(1536, 2048), np.arange(1024, 1536)])
    Wx = np.asarray(Wx, np.float32)[:, gperm]
    Wh = np.asarray(Wh, np.float32)[:, gperm]
    # embT [128, (kc4, t, b)]
    embT = np.zeros((128, 4 * T * Bc), np.float32)
    for kc in range(4):
        blk = emb_b[:, :, kc * 128:(kc + 1) * 128]      # (T, Bc, 128)
        embT[:, kc * T * Bc:(kc + 1) * T * Bc] = blk.transpose(2, 0, 1).reshape(128, T * Bc)
    # Wg: x-part (4 chunks) + h-part (4 chunks); rvec-part separate (Wgr)
    Wg = np.zeros((128, 8 * 2048), np.float32)
    for kc in range(4):
        Wg[:, kc * 2048:(kc + 1) * 2048] = Wx[kc * 128:(kc + 1) * 128]
    for kc in range(4):
        Wg[:, (4 + kc) * 2048:(5 + kc) * 2048] = Wh[kc * 128:(kc + 1) * 128]
    Wgr = np.zeros((64, 4 * 2048), np.float32)
    for r in range(4):
        Wgr[:, r * 2048:(r + 1) * 2048] = Wx[512 + r * 64:512 + (r + 1) * 64]
    Wif = np.zeros((128, 4 * IF), np.float32)
    for kc in range(4):
        Wif[:, kc * IF:(kc + 1) * IF] = Wif_p[kc * 128:(kc + 1) * 128]
    Wp = np.zeros((128, 4 * 512), np.float32)
    for kc in range(4):
        Wp[:, kc * 512:(kc + 1) * 512] = W_pre[kc * 128:(kc + 1) * 128]
    Wr = np.zeros((64, 4 * 512), np.float32)
    for r in range(4):
        Wr[:, r * 512:(r + 1) * 512] = W_rout[r * 64:(r + 1) * 64]
    consts = np.zeros((128, 16), np.float32)
    iota = np.arange(N, dtype=np.float32)
    consts[:, 0] = 1.0 + iota[:128] * np.float32(4e-7)   # PA chunk0
    consts[:, 1] = 1.0 + iota[128:] * np.float32(4e-7)   # PA chunk1
    consts[:, 2] = iota[:128] * np.float32(1e-30)
    consts[:, 3] = iota[128:] * np.float32(1e-30)
    consts[:, 4] = -1.0
    consts[:, 5] = 1.0
    consts[:64, 6] = 1.0     # e0: rows < 64
    consts[64:, 7] = 1.0     # e1: rows >= 64
    Wgr_p = np.zeros((128, Wgr.shape[1]), np.float32); Wgr_p[:64] = Wgr
    Wr_p = np.zeros((128, Wr.shape[1]), np.float32); Wr_p[:64] = Wr
    es = np.zeros((128, 1024), np.float32)
    es[:8] = np.repeat(np.eye(8, dtype=np.float32), 128, axis=1)
    blob = np.concatenate([Wg, Wgr_p, Wif, Wp, Wr_p, consts,
                           np.eye(128, dtype=np.float32), es, embT], axis=1)
    return {"blob": blob}


def kernel(emb_utt, Wx, Wh, b_lstm, W_pre, b_pre, W_iface, b_iface, W_rout):
    """Full-input entry point. Shards batch over 8 cores, runs the Bass kernel."""
    T, B = emb_utt.shape[0], emb_utt.shape[1]
    Bc = B // NCORES
    assert Bc == 4 and T == T_FULL
    # biases are all zero in this problem's setup_inputs; fold nonzero ones into
    # the weights via a constant-one input row would go here if ever needed.
    assert np.abs(b_lstm).max() == 0 and np.abs(b_pre).max() == 0 and np.abs(b_iface).max() == 0

    nc = bass.Bass("TRN2", target_bir_lowering=False, debug=False,
                   enable_asserts=False, num_devices=NCORES)
    build(nc, T, Bc)
    in_maps = []
    for core in range(NCORES):
        emb_b = np.asarray(emb_utt[:, core * Bc:(core + 1) * Bc], np.float32)
        in_maps.append(_prep_core_inputs(emb_b, np.asarray(Wx), np.asarray(Wh),
                                         np.asarray(W_pre), np.asarray(W_iface),
                                         np.asarray(W_rout), T, Bc))
    res = run_bass_kernel_spmd(nc, in_maps, list(range(NCORES)))
    global LAST_EXEC_NS
    LAST_EXEC_NS = res.exec_time_ns
    out = np.zeros((T, B, D), np.float32)
    for core in range(NCORES):
        out[:, core * Bc:(core + 1) * Bc] = res.results[core]["outs"]
    return out

